# revision 1
# baseline (speedup 1.0000x reference)
"""LSTM kernel for Trainium2 (Bass/Tile), 8-core data-parallel.

Model (per reference):
    xg = einsum('bsd,dg->sbg', x, Wi)            # input projections
    per step: z = xg_t + h @ Wh + bh
              i,f,g,o = split(z); c = sig(f)*c + sig(i)*tanh(g); h = sig(o)*tanh(c)
    out = h_last @ Wo + bo

Sharding: batch 256 -> 32 per core, weights replicated.

On-chip layout (per core):
  - gates-on-partitions: z for one step is a PSUM region [128, 128] laid out as
    [i|f|o|g] x 32 batch columns. Partition p = hidden feature; so i,f,o,g,c,h
    are all [H=128, B=32] tiles and h is directly the next matmul's rhs.
  - xg is precomputed by PE matmuls (lhsT = [Wi; bh] with a ones-row appended to
    x) straight into PSUM chunks of 16 steps; the per-step recurrence matmuls
    accumulate on top with start=False.
"""

import copy

import numpy as np

import concourse.bass as bass
import concourse.mybir as mybir
from concourse import tile
from concourse.bass_utils import run_bass_kernel_spmd

F32 = mybir.dt.float32

B, S, D, H = 256, 4096, 64, 128
G4 = 4 * H  # 512
NCORES = 8
BC = B // NCORES  # 32 batch per core
TC = 16  # timesteps per PSUM chunk (4 banks)
BODY_CH = 4  # chunks per loop body (static x-slot / psum ping-pong)
KD = D + 1  # contraction rows for input projection (ones row folds bh in)
CPC = TC * BC  # x columns per chunk (512)

# on-chip gate block order [i, f, o, g]; reference order is [i, f, g, o]
_PERM = np.concatenate(
    [np.arange(0, 128), np.arange(128, 256), np.arange(384, 512), np.arange(256, 384)]
)


def _legalize_for_walrus(nc):
    """Make the Tile-scheduled module lowerable by this walrus build.

    (1) This walrus accepts only ONE semaphore wait per TPB instruction
        (e.g. Matmult/LDWEIGHTS and DMACopy structs have a single wait slot);
        Tile emits multi-wait instructions. Hoist excess waits onto standalone
        EventSemaphore sequencer instructions placed just before, on the same
        engine — semantically identical (the sequencer blocks in order).
    (2) Drop the trailing EVENT_SEMAPHORE_RANGE_CLEAR InstISA (sem-recycling
        hygiene) which this walrus cannot lower at all.
    """
    f = nc.m.functions[0]
    template = None
    for blk in f.blocks:
        for inst in blk.instructions:
            if type(inst).__name__ == "InstEventSemaphore":
                template = inst
                break
        if template is not None:
            break
    assert template is not None, "no EventSemaphore to clone"
    uid = 0
    for blk in f.blocks:
        out = []
        for inst in blk.instructions:
            nm = type(inst).__name__
            if nm == "InstISA":
                continue  # (2)
            si = inst.sync_info
            waits = list(si.on_wait) if si is not None else []
            if nm != "InstEventSemaphore" and len(waits) > 1:
                for w in waits[1:]:
                    es = copy.deepcopy(template)
                    es.name = f"{inst.name}_hoist{uid}"
                    uid += 1
                    es.engine = inst.engine
                    es.sync_info = mybir.SyncInfo(on_wait=[w], on_update=[])
                    out.append(es)
                inst.sync_info = mybir.SyncInfo(
                    on_wait=waits[:1], on_update=list(si.on_update)
                )
            out.append(inst)
        blk.instructions = out


def build_bass(n_steps=S, legalize=True):
    n_ch = n_steps // TC
    assert n_ch % BODY_CH == 0 and n_steps % TC == 0
    n_iter = n_ch // BODY_CH
    pad_ch = n_ch + BODY_CH
    xcols = pad_ch * CPC

    nc = bass.Bass()
    xt = nc.declare_dram_parameter("xt", [KD, xcols], F32, isOutput=False)
    # combined weights: cols [0:512] = Wh (permuted), cols [512:1024] = [Wi; bh]
    # (rows 65:128 of the right half are zero padding)
    wcb = nc.declare_dram_parameter("wcb", [H, 2 * G4], F32, isOutput=False)
    hout = nc.declare_dram_parameter("h_out", [H, BC], F32, isOutput=True)

    with tile.TileContext(nc) as tc:
        with (
            tc.tile_pool(name="weights", bufs=1) as wpool,
            tc.tile_pool(name="xin", bufs=1) as xpool,
            tc.tile_pool(name="state", bufs=1) as spool,
            tc.tile_pool(name="psum", bufs=1, space=bass.MemorySpace.PSUM) as ppool,
        ):
            w_sb = wpool.tile([H, 2 * G4], F32, tag="w")
            wh_sb = w_sb[:, 0:G4]
            wi_sb = w_sb[:KD, G4 : 2 * G4]
            xs_all = xpool.tile([KD, BODY_CH * CPC], F32, tag="xs")
            xs = [xs_all[:, k * CPC : (k + 1) * CPC] for k in range(BODY_CH)]
            # persistent state: [i|f|o|g|c] so that [i|f] and [g|c] are each
            # contiguous 64-col spans (one fused tensor_tensor covers u=i*g, v=f*c)
            st = spool.tile([H, 160], F32, tag="st")
            wk = spool.tile([H, 96], F32, tag="wk")  # [u|v|tanh_c]
            h_sb = spool.tile([H, BC], F32, tag="h")
            ps = [
                ppool.tile([H, TC * 128], F32, tag=f"ps{k}", name=f"ps{k}")
                for k in range(2)
            ]

            # chunk layout per psum tile: [bank q (4)][gate block gb (4)][t (4)][b (32)]
            # so each xg matmul writes one contiguous [128, 128] in-bank region.
            def xg_chunk(p, xsrc):
                """Input-projection matmuls for one 16-step chunk into psum tile p."""
                for gb in range(4):
                    lhsT = wi_sb[:, gb * H : (gb + 1) * H]
                    for q in range(TC // 4):  # one matmul per PSUM bank
                        nc.tensor.matmul(
                            p[:, q * 512 + gb * 128 : q * 512 + (gb + 1) * 128],
                            lhsT,
                            xsrc[:, q * 4 * BC : (q + 1) * 4 * BC],
                            start=(gb == 0),
                            stop=False,
                            skip_group_check=True,
                        )

            def step(p, j):
                """One LSTM timestep; z for step j=4q+r is strided inside bank q."""
                q, r = j // 4, j % 4
                zoff = q * 512 + r * BC
                for gb in range(4):
                    nc.tensor.matmul(
                        p[:, zoff + gb * 128 : zoff + gb * 128 + BC],
                        wh_sb[:, gb * H : (gb + 1) * H],
                        h_sb[:, :],
                        start=False,
                        stop=True,
                        skip_group_check=True,
                    )
                act = mybir.ActivationFunctionType
                # strided views: gates i,f,o (and g) for step j sit 128 apart
                pz = p[:].rearrange("p (q gb z) -> p q gb z", q=4, gb=4)[:, q, :, :]
                # sigmoid over [i|f|o], tanh over g (PSUM -> SBUF)
                nc.scalar.activation(
                    st[:].rearrange("p (a z) -> p a z", z=BC)[:, 0:3, :],
                    pz[:, 0:3, r * BC : (r + 1) * BC],
                    act.Sigmoid,
                )
                nc.scalar.activation(
                    st[:, 96:128], pz[:, 3, r * BC : (r + 1) * BC], act.Tanh
                )
                # [u|v] = [i|f] * [g|c]
                nc.vector.tensor_mul(wk[:, 0:64], st[:, 0:64], st[:, 96:160])
                # c = u + v
                nc.vector.tensor_add(st[:, 128:160], wk[:, 0:32], wk[:, 32:64])
                nc.scalar.activation(wk[:, 64:96], st[:, 128:160], act.Tanh)
                # h = o * tanh(c)
                nc.vector.tensor_mul(h_sb[:, :], st[:, 64:96], wk[:, 64:96])

            def rec_chunk(p):
                for j in range(TC):
                    step(p, j)

            # ---- preamble ----
            nc.sync.dma_start(w_sb[:], wcb[:])
            nc.vector.memset(h_sb[:], 0.0)
            nc.vector.memset(st[:, 128:160], 0.0)  # c = 0
            nc.sync.dma_start(xs_all[:], xt[:, 0 : BODY_CH * CPC])
            xg_chunk(ps[0], xs[0])
            xg_chunk(ps[1], xs[1])

            # ---- main loop: body covers chunks 4i .. 4i+3 ----
            with tc.For_i(
                0, n_iter, 1, hint_engines=(mybir.EngineType.PE,)
            ) as iv:
                base = iv * (BODY_CH * CPC)

                rec_chunk(ps[0])        # chunk 4i
                xg_chunk(ps[0], xs[2])  # chunk 4i+2
                rec_chunk(ps[1])        # chunk 4i+1
                xg_chunk(ps[1], xs[3])  # chunk 4i+3
                # one DMA refills all four slots (chunks 4i+4 .. 4i+7); its WAR
                # on the slot-2/3 reads above orders it mid-body automatically
                nc.sync.dma_start(
                    xs_all[:], xt[:, bass.ds(base + BODY_CH * CPC, BODY_CH * CPC)]
                )
                rec_chunk(ps[0])        # chunk 4i+2
                xg_chunk(ps[0], xs[0])  # chunk 4i+4
                rec_chunk(ps[1])        # chunk 4i+3
                xg_chunk(ps[1], xs[1])  # chunk 4i+5

            nc.sync.dma_start(hout[:], h_sb[:])

    if legalize:  # CoreSim can't run the post-hoc clones; HW compile needs them
        _legalize_for_walrus(nc)
    return nc


def host_inputs(x, Wi, Wh, bh, n_steps=S):
    """Per-core input maps: transposed/padded x, permuted weights."""
    n_ch = n_steps // TC
    pad_ch = n_ch + BODY_CH
    xcols = pad_ch * CPC
    wcb = np.zeros((H, 2 * G4), np.float32)
    wcb[:, 0:G4] = Wh[:, _PERM]
    wcb[0:D, G4:] = Wi[:, _PERM]
    wcb[D, G4:] = bh[_PERM]
    nb = x.shape[0] // NCORES
    in_maps = []
    for core in range(NCORES):
        xc = x[core * nb : (core + 1) * nb]  # [BC, n_steps, D]
        xtc = np.ascontiguousarray(xc.transpose(2, 1, 0)).reshape(D, n_steps * nb)
        full = np.zeros((KD, xcols), np.float32)
        full[:D, : n_steps * nb] = xtc
        full[D, :] = 1.0
        in_maps.append({"xt": full, "wcb": wcb})
    return in_maps


_CACHE = {}


def _run(x, Wi, Wh, bh, trace=False):
    x = np.asarray(x, np.float32)
    if "nc" not in _CACHE:
        _CACHE["nc"] = build_bass()
    nc = _CACHE["nc"]
    in_maps = host_inputs(x, Wi, Wh, bh)
    res = run_bass_kernel_spmd(nc, in_maps, list(range(NCORES)), trace=trace)
    h_full = np.concatenate(
        [np.asarray(res.results[c]["h_out"]).T for c in range(NCORES)], axis=0
    )  # [B, H]
    return h_full, res


def kernel(x, Wi, Wh, bh, Wo, bo):
    x = np.asarray(x, np.float32)
    Wi = np.asarray(Wi, np.float32)
    Wh = np.asarray(Wh, np.float32)
    bh = np.asarray(bh, np.float32)
    Wo = np.asarray(Wo, np.float32)
    bo = np.asarray(bo, np.float32)
    h_full, _ = _run(x, Wi, Wh, bh)
    return (h_full @ Wo + bo).astype(np.float32)



# revision 2
# speedup vs baseline: 1.1252x; 1.1252x over previous
"""LSTM kernel for Trainium2 (Bass/Tile), 8-core data-parallel.

Model (per reference):
    xg = einsum('bsd,dg->sbg', x, Wi)            # input projections
    per step: z = xg_t + h @ Wh + bh
              i,f,g,o = split(z); c = sig(f)*c + sig(i)*tanh(g); h = sig(o)*tanh(c)
    out = h_last @ Wo + bo
Sharding: batch 256 -> 32 per core, weights replicated.

On-chip layout (per core):
  - gates-on-partitions: z for one step is a PSUM region [128, 128] laid out as
    [i|f|o|g] x 32 batch columns. Partition p = hidden feature; so i,f,o,g,c,h
    are all [H=128, B=32] tiles and h is directly the next matmul's rhs.
  - xg is precomputed by PE matmuls (lhsT = [Wi; bh] with a ones-row appended to
    x) straight into PSUM chunks of 16 steps; the per-step recurrence matmuls
    accumulate on top with start=False.
  - all matmul operands are bf16 (1 cycle/row on PE vs 4 for fp32); PSUM fp32.
  - the g-gate weights are pre-scaled by 2 on the host so tanh(zg) =
    2*sigmoid(2*zg) - 1 comes out of the SAME sigmoid activation as i,f,o
    (one ACT instr over all 128 gate columns); the affine fixup folds into
    the DVE ops:  u = (g' - 0.5)*i ;  c = 2*u + f*c  (scalar_tensor_tensor).
"""

import copy

import numpy as np

import concourse.bass as bass
import concourse.mybir as mybir
from concourse import tile
from concourse.bass_utils import run_bass_kernel_spmd

F32 = mybir.dt.float32
BF16 = mybir.dt.bfloat16
NP_BF16 = mybir.dt.np(mybir.dt.bfloat16)

B, S, D, H = 256, 4096, 64, 128
G4 = 4 * H  # 512
NCORES = 8
BC = B // NCORES  # 32 batch per core
TC = 16  # timesteps per PSUM chunk (4 banks)
BODY_CH = 4  # chunks per loop body (static x-slot / psum ping-pong)
KD = D + 1  # contraction rows for input projection (ones row folds bh in)
CPC = TC * BC  # x columns per chunk (512)

# on-chip gate block order [i, f, o, g]; reference order is [i, f, g, o]
_PERM = np.concatenate(
    [np.arange(0, 128), np.arange(128, 256), np.arange(384, 512), np.arange(256, 384)]
)


def _legalize_for_walrus(nc):
    """Make the Tile-scheduled module lowerable by this walrus build.

    (1) This walrus accepts only ONE semaphore wait per TPB instruction
        (e.g. Matmult/LDWEIGHTS and DMACopy structs have a single wait slot);
        Tile emits multi-wait instructions. Hoist excess waits onto standalone
        EventSemaphore sequencer instructions placed just before, on the same
        engine — semantically identical (the sequencer blocks in order).
    (2) Drop the trailing EVENT_SEMAPHORE_RANGE_CLEAR InstISA (sem-recycling
        hygiene) which this walrus cannot lower at all.
    """
    f = nc.m.functions[0]
    template = None
    for blk in f.blocks:
        for inst in blk.instructions:
            if type(inst).__name__ == "InstEventSemaphore":
                template = inst
                break
        if template is not None:
            break
    assert template is not None, "no EventSemaphore to clone"
    uid = 0
    for blk in f.blocks:
        out = []
        for inst in blk.instructions:
            nm = type(inst).__name__
            if nm == "InstISA":
                continue  # (2)
            si = inst.sync_info
            waits = list(si.on_wait) if si is not None else []
            if nm != "InstEventSemaphore" and len(waits) > 1:
                for w in waits[1:]:
                    es = copy.deepcopy(template)
                    es.name = f"{inst.name}_hoist{uid}"
                    uid += 1
                    es.engine = inst.engine
                    es.sync_info = mybir.SyncInfo(on_wait=[w], on_update=[])
                    out.append(es)
                inst.sync_info = mybir.SyncInfo(
                    on_wait=waits[:1], on_update=list(si.on_update)
                )
            out.append(inst)
        blk.instructions = out


def build_bass(n_steps=S, legalize=True):
    n_ch = n_steps // TC
    assert n_ch % BODY_CH == 0 and n_steps % TC == 0
    n_iter = n_ch // BODY_CH
    pad_ch = n_ch + BODY_CH
    xcols = pad_ch * CPC

    nc = bass.Bass()
    xt = nc.declare_dram_parameter("xt", [KD, xcols], BF16, isOutput=False)
    # combined weights: cols [0:512] = Wh (permuted), cols [512:1024] = [Wi; bh]
    # (rows 65:128 of the right half are zero padding); g blocks pre-scaled x2
    wcb = nc.declare_dram_parameter("wcb", [H, 2 * G4], BF16, isOutput=False)
    hout = nc.declare_dram_parameter("h_out", [H, BC], F32, isOutput=True)

    with tile.TileContext(nc) as tc:
        with (
            tc.tile_pool(name="weights", bufs=1) as wpool,
            tc.tile_pool(name="xin", bufs=1) as xpool,
            tc.tile_pool(name="state", bufs=1) as spool,
            tc.tile_pool(name="psum", bufs=1, space=bass.MemorySpace.PSUM) as ppool,
        ):
            w_sb = wpool.tile([H, 2 * G4], BF16, tag="w")
            wh_sb = w_sb[:, 0:G4]
            wi_sb = w_sb[:KD, G4 : 2 * G4]
            xs_all = xpool.tile([KD, BODY_CH * CPC], BF16, tag="xs")
            xs = [xs_all[:, k * CPC : (k + 1) * CPC] for k in range(BODY_CH)]
            # persistent state: st = sigmoid outputs [i|f|o|g'] (bf16),
            # cc = cell state (fp32), wk = [u|v] (fp32), tc_sb = tanh(c) (bf16)
            st = spool.tile([H, 4 * BC], BF16, tag="st")
            cc = spool.tile([H, BC], F32, tag="cc")
            wk = spool.tile([H, 2 * BC], F32, tag="wk")
            tc_sb = spool.tile([H, BC], BF16, tag="tc")
            h_sb = spool.tile([H, BC], BF16, tag="h")
            hf_sb = spool.tile([H, BC], F32, tag="hf")
            ps = [
                ppool.tile([H, TC * 128], F32, tag=f"ps{k}", name=f"ps{k}")
                for k in range(2)
            ]

            # chunk layout per psum tile: [bank q (4)][gate block gb (4)][t (4)][b (32)]
            # so each xg matmul writes one contiguous [128, 128] in-bank region.
            def xg_chunk(p, xsrc):
                """Input-projection matmuls for one 16-step chunk into psum tile p."""
                for gb in range(4):
                    lhsT = wi_sb[:, gb * H : (gb + 1) * H]
                    for q in range(TC // 4):  # one matmul per PSUM bank
                        nc.tensor.matmul(
                            p[:, q * 512 + gb * 128 : q * 512 + (gb + 1) * 128],
                            lhsT,
                            xsrc[:, q * 4 * BC : (q + 1) * 4 * BC],
                            start=(gb == 0),
                            stop=False,
                            skip_group_check=True,
                        )

            def step(p, j):
                """One LSTM timestep; z for step j=4q+r is strided inside bank q."""
                q, r = j // 4, j % 4
                zoff = q * 512 + r * BC
                for gb in range(4):
                    nc.tensor.matmul(
                        p[:, zoff + gb * 128 : zoff + gb * 128 + BC],
                        wh_sb[:, gb * H : (gb + 1) * H],
                        h_sb[:, :],
                        start=False,
                        stop=True,
                        skip_group_check=True,
                    )
                act = mybir.ActivationFunctionType
                alu = mybir.AluOpType
                # strided view: all four gate blocks for step j sit 128 apart
                pz = p[:].rearrange("p (q gb z) -> p q gb z", q=4, gb=4)[:, q, :, :]
                # one sigmoid over [i|f|o|g'] (g weights pre-scaled x2 on host)
                nc.scalar.activation(
                    st[:].rearrange("p (a z) -> p a z", z=BC)[:, :, :],
                    pz[:, :, r * BC : (r + 1) * BC],
                    act.Sigmoid,
                )
                # v = f * c  (reads previous c)
                nc.vector.tensor_mul(wk[:, BC : 2 * BC], st[:, BC : 2 * BC], cc[:, :])
                # u = (g' - 0.5) * i   [= 0.5 * i * tanh(zg)]
                nc.vector.scalar_tensor_tensor(
                    wk[:, 0:BC], st[:, 3 * BC : 4 * BC], 0.5, st[:, 0:BC],
                    alu.subtract, alu.mult,
                )
                # c = 2*u + v
                nc.vector.scalar_tensor_tensor(
                    cc[:, :], wk[:, 0:BC], 2.0, wk[:, BC : 2 * BC],
                    alu.mult, alu.add,
                )
                nc.scalar.activation(tc_sb[:, :], cc[:, :], act.Tanh)
                # h = o * tanh(c)
                nc.vector.tensor_mul(h_sb[:, :], st[:, 2 * BC : 3 * BC], tc_sb[:, :])

            def rec_chunk(p):
                for j in range(TC):
                    step(p, j)

            # ---- preamble ----
            nc.sync.dma_start(w_sb[:], wcb[:])
            nc.vector.memset(h_sb[:], 0.0)
            nc.vector.memset(cc[:], 0.0)
            nc.sync.dma_start(xs_all[:], xt[:, 0 : BODY_CH * CPC])
            xg_chunk(ps[0], xs[0])
            xg_chunk(ps[1], xs[1])

            # ---- main loop: body covers chunks 4i .. 4i+3 ----
            with tc.For_i(
                0, n_iter, 1, hint_engines=(mybir.EngineType.PE,)
            ) as iv:
                base = iv * (BODY_CH * CPC)

                rec_chunk(ps[0])        # chunk 4i
                xg_chunk(ps[0], xs[2])  # chunk 4i+2
                rec_chunk(ps[1])        # chunk 4i+1
                xg_chunk(ps[1], xs[3])  # chunk 4i+3
                # one DMA refills all four slots (chunks 4i+4 .. 4i+7); its WAR
                # on the slot-2/3 reads above orders it mid-body automatically
                nc.sync.dma_start(
                    xs_all[:], xt[:, bass.ds(base + BODY_CH * CPC, BODY_CH * CPC)]
                )
                rec_chunk(ps[0])        # chunk 4i+2
                xg_chunk(ps[0], xs[0])  # chunk 4i+4
                rec_chunk(ps[1])        # chunk 4i+3
                xg_chunk(ps[1], xs[1])  # chunk 4i+5

            # widen the final h to fp32 for the output DMA
            nc.vector.tensor_scalar_add(hf_sb[:, :], h_sb[:, :], 0.0)
            nc.sync.dma_start(hout[:], hf_sb[:])

    if legalize:  # CoreSim can't run the post-hoc clones; HW compile needs them
        _legalize_for_walrus(nc)
    return nc


def host_inputs(x, Wi, Wh, bh, n_steps=S):
    """Per-core input maps: transposed/padded x (bf16), permuted weights (bf16,
    g block pre-scaled by 2 for the tanh-via-sigmoid trick)."""
    n_ch = n_steps // TC
    pad_ch = n_ch + BODY_CH
    xcols = pad_ch * CPC
    gscale = np.ones((G4,), np.float32)
    gscale[384:512] = 2.0  # post-perm cols 384:512 are the g block
    wcb = np.zeros((H, 2 * G4), np.float32)
    wcb[:, 0:G4] = Wh[:, _PERM] * gscale
    wcb[0:D, G4:] = Wi[:, _PERM] * gscale
    wcb[D, G4:] = bh[_PERM] * gscale
    wcb = wcb.astype(NP_BF16)
    nb = x.shape[0] // NCORES
    in_maps = []
    for core in range(NCORES):
        xc = x[core * nb : (core + 1) * nb]  # [BC, n_steps, D]
        xtc = np.ascontiguousarray(xc.transpose(2, 1, 0)).reshape(D, n_steps * nb)
        full = np.zeros((KD, xcols), NP_BF16)
        full[:D, : n_steps * nb] = xtc.astype(NP_BF16)
        full[D, :] = 1.0
        in_maps.append({"xt": full, "wcb": wcb})
    return in_maps


_CACHE = {}


def _run(x, Wi, Wh, bh, trace=False):
    x = np.asarray(x, np.float32)
    if "nc" not in _CACHE:
        _CACHE["nc"] = build_bass()
    nc = _CACHE["nc"]
    in_maps = host_inputs(x, Wi, Wh, bh)
    res = run_bass_kernel_spmd(nc, in_maps, list(range(NCORES)), trace=trace)
    h_full = np.concatenate(
        [np.asarray(res.results[c]["h_out"]).astype(np.float32).T for c in range(NCORES)],
        axis=0,
    )  # [B, H]
    return h_full, res


def kernel(x, Wi, Wh, bh, Wo, bo):
    x = np.asarray(x, np.float32)
    Wi = np.asarray(Wi, np.float32)
    Wh = np.asarray(Wh, np.float32)
    bh = np.asarray(bh, np.float32)
    Wo = np.asarray(Wo, np.float32)
    bo = np.asarray(bo, np.float32)
    h_full, _ = _run(x, Wi, Wh, bh)
    return (h_full @ Wo + bo).astype(np.float32)
